# revision 11
# baseline (speedup 1.0000x reference)
"""Trainium2 Bass kernel for 3-layer GCN link prediction (nn_GNet_77051713290258).

Strategy (1D graph parallel, 8 cores):
- Nodes sharded 6250/core; edges partitioned by destination node, sorted by dst.
- GCNConv out = D^-1/2 (A+I) D^-1/2 (x W) + b.  We use associativity:
  layer1 aggregates x first (128-dim) then applies W1; layers 2/3 apply W first
  (128/64-dim) then aggregate.  Source-side deg scaling is folded into the
  aggregated table (pre-scaled by dis[v]); dst-side scaling folds into the
  PSUM->SBUF copy on the scalar engine.
- Aggregation = indirect-DMA gather of source rows (batched, one DMA per dst
  tile) + one-hot selection matrices (DVE is_equal vs iota) + PE matmuls
  accumulating into PSUM (f32).  Data in bf16, accumulation f32.
- Per-layer AllGather of the (pre-scaled) post-matmul features, then decode
  gathers h3 endpoints and dots them.
"""

import os

import ml_dtypes
import numpy as np

N = 50000
M = 8          # cores
NS = N // M    # 6250 nodes per shard
P = 128
NT = (NS + P - 1) // P  # 49 dst tiles per core (48 full + 106)
F0, F1, F2, F3 = 128, 256, 128, 64
E_TEST = 400000
ET_CORE = E_TEST // M          # 50000
TC = (ET_CORE + P - 1) // P    # 391 test chunks per core
DECODE_BATCH = 98

BF16 = ml_dtypes.bfloat16

# module-level stash for test.py introspection
LAST_RESULTS = None


def _preprocess(x, train_edge, pos_edge, neg_edge):
    """Host-side: degrees, edge sort by dst, per-core chunked gather schedules."""
    src = train_edge[0].astype(np.int64)
    dst = train_edge[1].astype(np.int64)

    deg = np.bincount(dst, minlength=N).astype(np.float32) + 1.0
    dis = (1.0 / np.sqrt(deg)).astype(np.float32)

    # add self-loops, sort all edges by destination
    s_all = np.concatenate([src, np.arange(N, dtype=np.int64)])
    d_all = np.concatenate([dst, np.arange(N, dtype=np.int64)])
    order = np.argsort(d_all, kind="stable")
    s_sorted = s_all[order]
    d_sorted = d_all[order]

    # per-core contiguous slices (d_sorted ascending)
    core_lo = np.searchsorted(d_sorted, np.arange(M) * NS)
    core_hi = np.searchsorted(d_sorted, (np.arange(M) + 1) * NS)

    # per-core per-tile edge counts -> shared chunk schedule C[t]
    cnts = np.zeros((M, NT), dtype=np.int64)
    tiles = []
    for k in range(M):
        s_k = s_sorted[core_lo[k]:core_hi[k]]
        dl_k = d_sorted[core_lo[k]:core_hi[k]] - k * NS
        tb = np.searchsorted(dl_k, np.arange(NT + 1) * P)
        cnts[k] = tb[1:] - tb[:-1]
        tiles.append((s_k, dl_k, tb))
    C = np.maximum(1, (cnts.max(axis=0) + P - 1) // P).astype(np.int64)  # chunks per tile
    off = np.zeros(NT, dtype=np.int64)
    off[1:] = np.cumsum(C)[:-1]
    SC = int(C.sum())

    srcidx = np.zeros((M, P, SC), dtype=np.int32)
    dstloc = np.full((M, P, SC), -1.0, dtype=np.float32)
    for k in range(M):
        s_k, dl_k, tb = tiles[k]
        for t in range(NT):
            cnt = tb[t + 1] - tb[t]
            ct = int(C[t])
            sbuf = np.zeros(ct * P, dtype=np.int32)
            dbuf = np.full(ct * P, -1.0, dtype=np.float32)
            sbuf[:cnt] = s_k[tb[t]:tb[t + 1]]
            dbuf[:cnt] = (dl_k[tb[t]:tb[t + 1]] - t * P).astype(np.float32)
            srcidx[k, :, off[t]:off[t] + ct] = sbuf.reshape(ct, P).T
            dstloc[k, :, off[t]:off[t] + ct] = dbuf.reshape(ct, P).T

    # test edges
    te = np.concatenate([pos_edge, neg_edge], axis=1).astype(np.int64)  # [2, 400000]
    tsrc = np.zeros((M, P, TC), dtype=np.int32)
    tdst = np.zeros((M, P, TC), dtype=np.int32)
    for k in range(M):
        a = np.zeros(TC * P, dtype=np.int32)
        b = np.zeros(TC * P, dtype=np.int32)
        a[:ET_CORE] = te[0, k * ET_CORE:(k + 1) * ET_CORE]
        b[:ET_CORE] = te[1, k * ET_CORE:(k + 1) * ET_CORE]
        tsrc[k] = a.reshape(TC, P).T
        tdst[k] = b.reshape(TC, P).T

    return dis, C, off, SC, srcidx, dstloc, tsrc, tdst


def _build_program(C, off, SC, b1_zero, b2_zero, b3_zero):
    from concourse import bacc, bass, mybir
    import concourse.tile as tile

    dt = mybir.dt
    DT = dt.bfloat16
    f32 = dt.float32
    i32 = dt.int32
    Cmax = int(C.max())

    nc = bacc.Bacc(None, target_bir_lowering=False, num_devices=M)

    # ---- I/O ----
    xg_d = nc.dram_tensor("xg", [P, SC * F0], DT, kind="ExternalInput")
    srcidx_d = nc.dram_tensor("srcidx", [P, SC], i32, kind="ExternalInput")
    dstloc_d = nc.dram_tensor("dstloc", [P, SC], DT, kind="ExternalInput")
    disp_d = nc.dram_tensor("disp", [NT * P, 1], f32, kind="ExternalInput")
    w1_d = nc.dram_tensor("w1", [F0, F1], DT, kind="ExternalInput")
    w2_d = nc.dram_tensor("w2", [F1, F2], DT, kind="ExternalInput")
    w3_d = nc.dram_tensor("w3", [F2, F3], DT, kind="ExternalInput")
    b1_d = nc.dram_tensor("b1", [F1, 1], f32, kind="ExternalInput")
    b2bc_d = nc.dram_tensor("b2bc", [P, F2], f32, kind="ExternalInput")
    b3bc_d = nc.dram_tensor("b3bc", [P, F3], f32, kind="ExternalInput")
    iota_d = nc.dram_tensor("iota", [P, P], DT, kind="ExternalInput")
    ident_d = nc.dram_tensor("ident", [P, P], f32, kind="ExternalInput")
    tsrc_d = nc.dram_tensor("tsrc", [P, TC], i32, kind="ExternalInput")
    tdst_d = nc.dram_tensor("tdst", [P, TC], i32, kind="ExternalInput")
    dot_d = nc.dram_tensor("dotout", [P, TC], f32, kind="ExternalOutput")

    # ---- internal DRAM (collectives must not touch I/O tensors) ----
    m2b_d = nc.dram_tensor("m2b", [NS, F2], DT)
    m2full_d = nc.dram_tensor("m2full", [N, F2], DT, addr_space="Shared")
    m3b_d = nc.dram_tensor("m3b", [NS, F3], DT)
    m3full_d = nc.dram_tensor("m3full", [N, F3], DT, addr_space="Shared")
    h3b_d = nc.dram_tensor("h3b", [NS, F3], DT)
    h3full_d = nc.dram_tensor("h3full", [N, F3], DT, addr_space="Shared")

    RG = [list(range(M))]
    eq = mybir.AluOpType.is_equal
    Relu = mybir.ActivationFunctionType.Relu
    Copy = mybir.ActivationFunctionType.Copy

    def rows(t):
        return min(P, NS - t * P)

    with tile.TileContext(nc) as tc:
        with (
            tc.tile_pool(name="const", bufs=1) as cp,
            tc.tile_pool(name="persist", bufs=1) as pp,
            tc.tile_pool(name="work", bufs=3) as wp,
            tc.tile_pool(name="small", bufs=4) as sp,
            tc.tile_pool(name="psum", bufs=2, space="PSUM") as ps,
        ):
            # constants
            iota_sb = cp.tile([P, P], DT, tag="iota")
            nc.sync.dma_start(out=iota_sb[:], in_=iota_d[:, :])
            ident_sb = cp.tile([P, P], f32, tag="ident")
            nc.sync.dma_start(out=ident_sb[:], in_=ident_d[:, :])
            w1_sb = cp.tile([F0, F1], DT, tag="w1")
            nc.sync.dma_start(out=w1_sb[:], in_=w1_d[:, :])
            w2a_sb = cp.tile([P, F2], DT, tag="w2a")
            nc.sync.dma_start(out=w2a_sb[:], in_=w2_d[0:P, :])
            w2b_sb = cp.tile([P, F2], DT, tag="w2b")
            nc.sync.dma_start(out=w2b_sb[:], in_=w2_d[P:F1, :])
            w3_sb = cp.tile([F2, F3], DT, tag="w3")
            nc.sync.dma_start(out=w3_sb[:], in_=w3_d[:, :])
            if not b1_zero:
                b1a_sb = cp.tile([P, 1], f32, tag="b1a")
                nc.sync.dma_start(out=b1a_sb[:], in_=b1_d[0:P, :])
                b1b_sb = cp.tile([P, 1], f32, tag="b1b")
                nc.sync.dma_start(out=b1b_sb[:], in_=b1_d[P:F1, :])
            if not b2_zero:
                b2bc_sb = cp.tile([P, F2], f32, tag="b2bc")
                nc.sync.dma_start(out=b2bc_sb[:], in_=b2bc_d[:, :])
            if not b3_zero:
                b3bc_sb = cp.tile([P, F3], f32, tag="b3bc")
                nc.sync.dma_start(out=b3bc_sb[:], in_=b3bc_d[:, :])

            h1Ta = pp.tile([P, NS], DT, tag="h1Ta")  # h1 features 0:128, feature-major
            h1Tb = pp.tile([P, NS], DT, tag="h1Tb")  # h1 features 128:256
            h2T = pp.tile([P, NS], DT, tag="h2T")    # h2 feature-major
            dot_sb = pp.tile([P, TC], f32, tag="dot")

            def load_tile_meta(t):
                ct = int(C[t])
                o = int(off[t])
                idx = sp.tile([P, Cmax], i32, tag="idx")
                nc.sync.dma_start(out=idx[:, :ct], in_=srcidx_d[:, o:o + ct])
                dloc = sp.tile([P, Cmax], DT, tag="dloc")
                nc.sync.dma_start(out=dloc[:, :ct], in_=dstloc_d[:, o:o + ct])
                dis_t = sp.tile([P, 1], f32, tag="dist")
                nc.sync.dma_start(out=dis_t[:], in_=disp_d[t * P:(t + 1) * P, :])
                return ct, idx, dloc, dis_t

            def aggregate(t, table_d, F, ct, idx, dloc, xg_d=None):
                """gather + one-hot matmul accumulate; returns psum [128, F] f32
                (rows beyond rows(t) are exact zeros).  HW indirect DMA supports
                one index per partition, so gathers are per-chunk; when xg_d is
                given (host pre-gathered messages) a single streaming DMA is
                used instead."""
                msg = wp.tile([P, Cmax * F], DT, tag=f"msg{F}")
                if xg_d is not None:
                    o = int(off[t])
                    nc.sync.dma_start(out=msg[:, :ct * F],
                                      in_=xg_d[:, o * F:(o + ct) * F])
                else:
                    for c in range(ct):
                        nc.gpsimd.indirect_dma_start(
                            out=msg[:, c * F:(c + 1) * F],
                            out_offset=None,
                            in_=table_d[:, :],
                            in_offset=bass.IndirectOffsetOnAxis(
                                ap=idx[:, c:c + 1], axis=0),
                        )
                oneh = wp.tile([P, Cmax * P], DT, tag="oneh")
                oneh3 = oneh[:, :ct * P].rearrange("p (c q) -> p c q", c=ct)
                nc.vector.tensor_tensor(
                    out=oneh3,
                    in0=dloc[:, :ct, None].to_broadcast((P, ct, P)),
                    in1=iota_sb[:, None, :].to_broadcast((P, ct, P)),
                    op=eq,
                )
                agg_ps = ps.tile([P, F], f32, tag="agg")
                for c in range(ct):
                    nc.tensor.matmul(
                        out=agg_ps[:],
                        lhsT=oneh[:, c * P:(c + 1) * P],
                        rhs=msg[:, c * F:(c + 1) * F],
                        start=(c == 0),
                        stop=(c == ct - 1),
                    )
                return agg_ps

            # =================== Layer 1 (+ m2 = h1 @ W2) ===================
            for t in range(NT):
                R = rows(t)
                ct, idx, dloc, dis_t = load_tile_meta(t)
                agg_ps = aggregate(t, None, F0, ct, idx, dloc, xg_d=xg_d)
                # dst-side deg scale, f32 sbuf node-major
                agg_sb = wp.tile([P, F0], f32, tag="aggsb")
                nc.scalar.activation(out=agg_sb[:], in_=agg_ps[:], func=Copy,
                                     scale=dis_t[:, 0:1])
                # transpose to feature-major
                tp_ps = ps.tile([P, P], f32, tag="tp")
                nc.tensor.transpose(out=tp_ps[:], in_=agg_sb[:], identity=ident_sb[:])
                aggT = wp.tile([P, P], DT, tag="aggT")
                nc.vector.tensor_copy(out=aggT[:], in_=tp_ps[:])
                # h1T = relu(W1^T aggT + b1), two 128-row halves
                for h, (h1T_h, bias_sb) in enumerate(
                    ((h1Ta, None if b1_zero else b1a_sb),
                     (h1Tb, None if b1_zero else b1b_sb))
                ):
                    hps = ps.tile([P, P], f32, tag="dense")
                    nc.tensor.matmul(
                        out=hps[:, :R],
                        lhsT=w1_sb[:, h * P:(h + 1) * P],
                        rhs=aggT[:, :R],
                        start=True, stop=True,
                    )
                    nc.scalar.activation(
                        out=h1T_h[:, t * P:t * P + R], in_=hps[:, :R], func=Relu,
                        bias=0.0 if bias_sb is None else bias_sb[:, 0:1])
                # m2 = h1 @ W2 (feature-major), K=256 in two chunks
                m2ps = ps.tile([P, P], f32, tag="dense")
                nc.tensor.matmul(out=m2ps[:, :R], lhsT=w2a_sb[:],
                                 rhs=h1Ta[:, t * P:t * P + R], start=True, stop=False)
                nc.tensor.matmul(out=m2ps[:, :R], lhsT=w2b_sb[:],
                                 rhs=h1Tb[:, t * P:t * P + R], start=False, stop=True)
                m2T_sb = wp.tile([P, P], f32, tag="m2T")
                nc.scalar.activation(out=m2T_sb[:, :R], in_=m2ps[:, :R], func=Copy)
                tp2_ps = ps.tile([P, P], f32, tag="tp")
                nc.tensor.transpose(out=tp2_ps[:R, :], in_=m2T_sb[:, :R],
                                    identity=ident_sb[:])
                m2_sb = wp.tile([P, F2], DT, tag="m2sb")
                nc.scalar.activation(out=m2_sb[:R, :], in_=tp2_ps[:R, :], func=Copy,
                                     scale=dis_t[:R, 0:1])
                nc.sync.dma_start(out=m2b_d[t * P:t * P + R, :], in_=m2_sb[:R, :])

            nc.gpsimd.collective_compute(
                "AllGather", mybir.AluOpType.bypass, replica_groups=RG,
                ins=[m2b_d[:, :].opt()], outs=[m2full_d[:, :].opt()],
            )

            # =================== Layer 2 aggregate (+ m3 = h2 @ W3) =========
            for t in range(NT):
                R = rows(t)
                ct, idx, dloc, dis_t = load_tile_meta(t)
                agg_ps = aggregate(t, m2full_d, F2, ct, idx, dloc)
                h2_sb = wp.tile([P, F2], f32, tag="h2sb")
                if b2_zero:
                    nc.scalar.activation(out=h2_sb[:], in_=agg_ps[:], func=Relu,
                                         scale=dis_t[:, 0:1])
                else:
                    nc.scalar.activation(out=h2_sb[:], in_=agg_ps[:], func=Copy,
                                         scale=dis_t[:, 0:1])
                    nc.vector.tensor_add(out=h2_sb[:], in0=h2_sb[:], in1=b2bc_sb[:])
                    nc.scalar.activation(out=h2_sb[:], in_=h2_sb[:], func=Relu)
                tp_ps = ps.tile([P, P], f32, tag="tp")
                nc.tensor.transpose(out=tp_ps[:], in_=h2_sb[:], identity=ident_sb[:])
                nc.vector.tensor_copy(out=h2T[:, t * P:t * P + R], in_=tp_ps[:, :R])
                # m3 = h2 @ W3 feature-major [64, R]
                m3ps = ps.tile([P, P], f32, tag="dense")
                nc.tensor.matmul(out=m3ps[:F3, :R], lhsT=w3_sb[:],
                                 rhs=h2T[:, t * P:t * P + R], start=True, stop=True)
                m3T_sb = wp.tile([F3, P], f32, tag="m3T")
                nc.scalar.activation(out=m3T_sb[:, :R], in_=m3ps[:F3, :R], func=Copy)
                tp3_ps = ps.tile([P, F3], f32, tag="tp")
                nc.tensor.transpose(out=tp3_ps[:R, :], in_=m3T_sb[:, :R],
                                    identity=ident_sb[:F3, :F3])
                m3_sb = wp.tile([P, F3], DT, tag="m3sb")
                nc.scalar.activation(out=m3_sb[:R, :], in_=tp3_ps[:R, :], func=Copy,
                                     scale=dis_t[:R, 0:1])
                nc.sync.dma_start(out=m3b_d[t * P:t * P + R, :], in_=m3_sb[:R, :])

            nc.gpsimd.collective_compute(
                "AllGather", mybir.AluOpType.bypass, replica_groups=RG,
                ins=[m3b_d[:, :].opt()], outs=[m3full_d[:, :].opt()],
            )

            # =================== Layer 3 aggregate ==========================
            for t in range(NT):
                R = rows(t)
                ct, idx, dloc, dis_t = load_tile_meta(t)
                agg_ps = aggregate(t, m3full_d, F3, ct, idx, dloc)
                h3_sb = wp.tile([P, F3], DT, tag="h3sb")
                if b3_zero:
                    nc.scalar.activation(out=h3_sb[:], in_=agg_ps[:], func=Copy,
                                         scale=dis_t[:, 0:1])
                else:
                    h3f_sb = wp.tile([P, F3], f32, tag="h3fsb")
                    nc.scalar.activation(out=h3f_sb[:], in_=agg_ps[:], func=Copy,
                                         scale=dis_t[:, 0:1])
                    nc.vector.tensor_add(out=h3f_sb[:], in0=h3f_sb[:], in1=b3bc_sb[:])
                    nc.vector.tensor_copy(out=h3_sb[:], in_=h3f_sb[:])
                nc.sync.dma_start(out=h3b_d[t * P:t * P + R, :], in_=h3_sb[:R, :])

            nc.gpsimd.collective_compute(
                "AllGather", mybir.AluOpType.bypass, replica_groups=RG,
                ins=[h3b_d[:, :].opt()], outs=[h3full_d[:, :].opt()],
            )

            # =================== Decode =====================================
            b0 = 0
            while b0 < TC:
                gb = min(DECODE_BATCH, TC - b0)
                tsa = sp.tile([P, DECODE_BATCH], i32, tag="tsa")
                nc.sync.dma_start(out=tsa[:, :gb], in_=tsrc_d[:, b0:b0 + gb])
                tsb = sp.tile([P, DECODE_BATCH], i32, tag="tsb")
                nc.sync.dma_start(out=tsb[:, :gb], in_=tdst_d[:, b0:b0 + gb])
                ga = wp.tile([P, DECODE_BATCH * F3], DT, tag="ga")
                gbt = wp.tile([P, DECODE_BATCH * F3], DT, tag="gb")
                for j in range(gb):
                    nc.gpsimd.indirect_dma_start(
                        out=ga[:, j * F3:(j + 1) * F3], out_offset=None,
                        in_=h3full_d[:, :],
                        in_offset=bass.IndirectOffsetOnAxis(ap=tsa[:, j:j + 1],
                                                            axis=0))
                    nc.gpsimd.indirect_dma_start(
                        out=gbt[:, j * F3:(j + 1) * F3], out_offset=None,
                        in_=h3full_d[:, :],
                        in_offset=bass.IndirectOffsetOnAxis(ap=tsb[:, j:j + 1],
                                                            axis=0))
                prod = wp.tile([P, DECODE_BATCH * F3], DT, tag="prod")
                nc.vector.tensor_tensor(out=prod[:, :gb * F3], in0=ga[:, :gb * F3],
                                        in1=gbt[:, :gb * F3], op=mybir.AluOpType.mult)
                nc.vector.tensor_reduce(
                    out=dot_sb[:, b0:b0 + gb],
                    in_=prod[:, :gb * F3].rearrange("p (c q) -> p c q", c=gb),
                    axis=mybir.AxisListType.X, op=mybir.AluOpType.add)
                b0 += gb
            nc.sync.dma_start(out=dot_d[:, :], in_=dot_sb[:])

    nc.compile()
    return nc


def _install_trace_shim():
    """Register the NTFF profile hook the image's antenv stub lacks (needed
    only when GNET_TRACE=1); neuter artifact upload (no object store here)."""
    import sys
    import types
    try:
        if "antenv.axon_hooks" not in sys.modules:
            mod = types.ModuleType("antenv.axon_hooks")
            store = {}
            mod.set_axon_ntff_profile_hook = lambda h: store.__setitem__("h", h)
            mod.get_axon_ntff_profile_hook = lambda: store.get("h")
            sys.modules["antenv.axon_hooks"] = mod
            import antenv
            antenv.axon_hooks = mod
            from trn_agent_boot.trn_boot import _ntff_profile_via_ctypes
            hook = _ntff_profile_via_ctypes("/opt/axon/libaxon_pjrt.so")
            if hook is not None:
                mod.set_axon_ntff_profile_hook(hook)
        from concourse import bass_utils
        bass_utils.upload_artifacts = lambda tmpdir: tmpdir
    except Exception:
        pass


def kernel(**inputs):
    global LAST_RESULTS
    from concourse import bass_utils

    x = np.asarray(inputs["x"], dtype=np.float32)
    train_edge = np.asarray(inputs["train_pos_edge_index"])
    pos_edge = np.asarray(inputs["pos_edge_index"])
    neg_edge = np.asarray(inputs["neg_edge_index"])
    W1 = np.asarray(inputs["W1"], dtype=np.float32)
    b1 = np.asarray(inputs["b1"], dtype=np.float32)
    W2 = np.asarray(inputs["W2"], dtype=np.float32)
    b2 = np.asarray(inputs["b2"], dtype=np.float32)
    W3 = np.asarray(inputs["W3"], dtype=np.float32)
    b3 = np.asarray(inputs["b3"], dtype=np.float32)

    dis, C, off, SC, srcidx, dstloc, tsrc, tdst = _preprocess(
        x, train_edge, pos_edge, neg_edge)

    xs = (x * dis[:, None]).astype(BF16)           # source-side pre-scale

    iota = np.broadcast_to(np.arange(P, dtype=np.float32), (P, P)).astype(BF16)
    ident = np.eye(P, dtype=np.float32)

    b1_zero = not np.any(b1)
    b2_zero = not np.any(b2)
    b3_zero = not np.any(b3)

    nc = _build_program(C, off, SC, b1_zero, b2_zero, b3_zero)

    in_maps = []
    for k in range(M):
        disp_k = np.zeros((NT * P, 1), dtype=np.float32)
        disp_k[:NS, 0] = dis[k * NS:(k + 1) * NS]
        in_maps.append({
            "xg": np.ascontiguousarray(xs[srcidx[k]].reshape(P, SC * F0)),
            "srcidx": np.ascontiguousarray(srcidx[k]),
            "dstloc": np.ascontiguousarray(dstloc[k].astype(BF16)),
            "disp": disp_k,
            "w1": W1.astype(BF16),
            "w2": W2.astype(BF16),
            "w3": W3.astype(BF16),
            "b1": b1.reshape(F1, 1).astype(np.float32),
            "b2bc": np.broadcast_to(b2, (P, F2)).astype(np.float32).copy(),
            "b3bc": np.broadcast_to(b3, (P, F3)).astype(np.float32).copy(),
            "iota": np.ascontiguousarray(iota),
            "ident": ident,
            "tsrc": np.ascontiguousarray(tsrc[k]),
            "tdst": np.ascontiguousarray(tdst[k]),
        })

    do_trace = bool(int(os.environ.get("GNET_TRACE", "0")))
    if do_trace:
        _install_trace_shim()
    res = bass_utils.run_bass_kernel_spmd(
        nc, in_maps, core_ids=list(range(M)), trace=do_trace,
    )
    LAST_RESULTS = res

    out = np.empty(E_TEST, dtype=np.float32)
    for k in range(M):
        r = res.results[k]["dotout"]          # [128, TC]
        out[k * ET_CORE:(k + 1) * ET_CORE] = r.T.reshape(-1)[:ET_CORE]
    return out
